# revision 15
# baseline (speedup 1.0000x reference)
"""Context2Query kernel for Trainium2 (8 NeuronCores, axon).

Computes: A = softmax(s, axis=1); out = (A @ u[0]).T   -> [D, T]

Sharding: T (context) axis split across 8 cores, 1024 rows each.

v2 design notes (per core):
  - s cast to fp16 on host (halves s DMA bytes; exp(s_fp16) rel err ~2e-3
    on E, RMS-cancels in the A@u sum -> output err ~1e-4 scale)
  - u repacked on host to [4 dcol-groups, 128, 16 jblk, 512 d] fp16 so
    each group of 4 output d-blocks needs only one contiguous 2MB DMA
  - t axis processed in 4 chunks of 256; per chunk: exp on ScalarE,
    PE-transpose (fp16, 1cyc/row) to E.T, copies PSUM->SBUF split
    between DVE (even k) and ScalarE (odd k, copy shares act table with
    exp so no table reload), denominator = fp16 pair-tree on DVE +
    single ones-matmul broadcast, main MMs m-grouped by dcol so u DMA
    arrival order matches compute order, out scale fused with
    PSUM->SBUF on DVE, per-m output DMA
  - PE warmup: dummy transposes on a memset tile from ~7us so the HAM
    clock gate opens (1.2->2.4GHz) before real work arrives
  - DMA rings: u + outputs on SP (sync) HWDGE ring, s + consts on ACT
    (scalar) ring; both FIFO per ring so issue order = drain order
"""

import time

import numpy as np
from contextlib import ExitStack

import concourse.bass as bass
import concourse.bacc as bacc
import concourse.mybir as mybir
from concourse.tile import TileContext
from concourse.bass_utils import run_bass_kernel_spmd

T, J, D = 8192, 2048, 2048
NCORES = 8
TLOC = T // NCORES   # 1024 context rows per core
C = 256              # t-chunk
NCH = TLOC // C      # 4 chunks
JB = J // 128        # 16 j-blocks
NG = 4               # d-column groups (4 m-blocks each)

F32 = mybir.dt.float32
F16 = mybir.dt.float16
AF = mybir.ActivationFunctionType

N_WARM = 20          # PE warmup transposes


def _build():
    nc = bacc.Bacc(trn_type="TRN2")

    s_dram = nc.dram_tensor("s_loc", [TLOC, J], F16, kind="ExternalInput").ap()
    u_dram = nc.dram_tensor("u_r", [NG, 128, JB, 512], F16, kind="ExternalInput").ap()
    i_dram = nc.dram_tensor("ident", [128, 128], F16, kind="ExternalInput").ap()
    w_dram = nc.dram_tensor("ones_m", [128, 128], F16, kind="ExternalInput").ap()
    o_dram = nc.dram_tensor("o_loc", [D, TLOC], F32, kind="ExternalOutput").ap()

    with TileContext(nc) as tc, ExitStack() as ctx:
        const_pool = ctx.enter_context(tc.tile_pool(name="const", bufs=1))
        s_pool = ctx.enter_context(tc.tile_pool(name="spool", bufs=16))
        u_pool = ctx.enter_context(tc.tile_pool(name="upool", bufs=1))
        an_pool = ctx.enter_context(tc.tile_pool(name="anpool", bufs=6))
        et_pool = ctx.enter_context(tc.tile_pool(name="etpool", bufs=3))
        ds_pool = ctx.enter_context(tc.tile_pool(name="dspool", bufs=10))
        rden_pool = ctx.enter_context(tc.tile_pool(name="rdenpool", bufs=2))
        osb_pool = ctx.enter_context(tc.tile_pool(name="osbpool", bufs=12))
        tp_psum = ctx.enter_context(tc.tile_pool(name="tppsum", bufs=2, space="PSUM"))
        den_psum = ctx.enter_context(tc.tile_pool(name="denpsum", bufs=1, space="PSUM"))
        out_psum = ctx.enter_context(tc.tile_pool(name="outpsum", bufs=5, space="PSUM"))

        # s c0 + consts on ACT ring (head priority, ring otherwise idle);
        # everything else on SP ring in exact need-order — FIFO per ring
        # means issue order == drain order, so arrival matches compute.
        s_tiles = {}

        def dma_s_chunk(c, eng):
            for h in range(2):
                for tb in (2 * c, 2 * c + 1):
                    st = s_pool.tile([128, 1024], F16, tag="s", name=f"s_{tb}_{h}")
                    eng.dma_start(
                        out=st,
                        in_=s_dram[tb * 128 : (tb + 1) * 128, h * 1024 : (h + 1) * 1024],
                    )
                    s_tiles[(tb, h)] = st

        # ACT ring: only the two h0 halves of s chunk 0, so the implicit
        # ACT_TABLE_LOAD (queued after these 2 issues) fires ASAP and the
        # first exp starts ~9.5us. Everything else need-ordered on SP.
        for tb in (0, 1):
            st = s_pool.tile([128, 1024], F16, tag="s", name=f"s_{tb}_0")
            nc.scalar.dma_start(out=st, in_=s_dram[tb * 128 : (tb + 1) * 128, 0:1024])
            s_tiles[(tb, 0)] = st
        for tb in (0, 1):
            st = s_pool.tile([128, 1024], F16, tag="s", name=f"s_{tb}_1")
            nc.sync.dma_start(out=st, in_=s_dram[tb * 128 : (tb + 1) * 128, 1024:2048])
            s_tiles[(tb, 1)] = st
        ident = const_pool.tile([128, 128], F16, name="ident_sb")
        nc.sync.dma_start(out=ident, in_=i_dram)
        ones_sb = const_pool.tile([128, 128], F16, name="ones_sb")
        nc.sync.dma_start(out=ones_sb, in_=w_dram)

        u_sb = []
        for g in range(NG):
            ut = u_pool.tile([128, JB, 512], F16, tag=f"u{g}", name=f"u{g}")
            u_sb.append(ut)
        nc.sync.dma_start(out=u_sb[0], in_=u_dram[0])
        dma_s_chunk(1, nc.sync)
        nc.sync.dma_start(out=u_sb[1], in_=u_dram[1])
        nc.sync.dma_start(out=u_sb[2], in_=u_dram[2])
        dma_s_chunk(2, nc.sync)
        nc.sync.dma_start(out=u_sb[3], in_=u_dram[3])
        dma_s_chunk(3, nc.sync)

        # PE warmup on a zeroed tile (DVE memset ~7us, then dummy
        # transposes keep the HAM activity window busy so the clock gate
        # opens before the real transposes arrive)
        warm_sb = const_pool.tile([128, 128], F16, name="warm_sb")
        nc.vector.memset(warm_sb, 0.0)
        for w in range(N_WARM):
            wt = tp_psum.tile([128, C], F16, tag="tp", name=f"warm_{w}")
            nc.tensor.transpose(wt[:, 0:128], warm_sb, warm_sb)

        a_nat = {}

        def exp_chunk(c):
            for tb in (2 * c, 2 * c + 1):
                a_nat[tb] = an_pool.tile([128, J], F16, tag="an", name=f"an_{tb}")
            for h in range(2):
                for tb in (2 * c, 2 * c + 1):
                    nc.scalar.activation(
                        a_nat[tb][:, h * 1024 : (h + 1) * 1024],
                        s_tiles[(tb, h)],
                        AF.Exp,
                    )

        et = {}

        def tr_k(c, k, dve_only=False):
            tp = tp_psum.tile([128, C], F16, tag="tp", name=f"tp_{c}_{k}")
            for i, tb in enumerate((2 * c, 2 * c + 1)):
                nc.tensor.transpose(
                    tp[:, i * 128 : (i + 1) * 128],
                    a_nat[tb][:, k * 128 : (k + 1) * 128],
                    ident,
                )
            if dve_only or k % 2 == 0:
                nc.vector.tensor_copy(et[c][:, k, :], tp)
            else:
                nc.scalar.activation(et[c][:, k, :], tp, AF.Copy)

        def tr_chunk(c):
            et[c] = et_pool.tile([128, JB, C], F16, tag="et", name=f"et_{c}")
            for k in range(JB):
                tr_k(c, k)

        def exp_tr_chunk0():
            # chunk 0 head path: 512-wide exp pieces jc-major, transposes
            # fire as soon as each jc column pair is exp'd
            for tb in (0, 1):
                a_nat[tb] = an_pool.tile([128, J], F16, tag="an", name=f"an_{tb}")
            et[0] = et_pool.tile([128, JB, C], F16, tag="et", name="et_0")
            for jc in range(4):
                for tb in (0, 1):
                    nc.scalar.activation(
                        a_nat[tb][:, jc * 512 : (jc + 1) * 512],
                        s_tiles[(tb, jc // 2)][:, (jc % 2) * 512 : ((jc % 2) + 1) * 512],
                        AF.Exp,
                    )
                for k in range(4 * jc, 4 * jc + 4):
                    tr_k(0, k, dve_only=True)  # ScalarE stays on exps

        def den_tree(c, eng):
            # chunk 0 on DVE (den_mm(0) is near the critical path); later
            # chunks on the idle GpSimd engine so DVE stays free for the
            # transpose-copy phases (which pace the PE via tp bufs=2)
            lvl = [et[c][:, k, :] for k in range(JB)]
            while len(lvl) > 1:
                nxt = []
                for i in range(0, len(lvl), 2):
                    t = ds_pool.tile([128, C], F16, tag="ds", name=f"ds_{c}_{len(lvl)}_{i}")
                    eng.tensor_add(t, lvl[i], lvl[i + 1])
                    nxt.append(t)
                lvl = nxt
            return lvl[0]

        roots = {}
        den_ps = {}
        rden = {}

        def den_mm(c):
            den_ps[c] = den_psum.tile([128, C], F32, tag="den", name=f"den_{c}")
            nc.tensor.matmul(den_ps[c], ones_sb, roots[c], start=True, stop=True)

        def recip(c):
            rden[c] = rden_pool.tile([128, C], F32, tag="rden", name=f"rden_{c}")
            nc.vector.reciprocal(rden[c], den_ps[c])

        ops_live = {}

        def mm_group(c, g):
            for mm in range(4):
                m = 4 * g + mm
                ops = out_psum.tile([128, C], F32, tag="ops", name=f"o_{c}_{m}")
                for k in range(JB):
                    nc.tensor.matmul(
                        ops,
                        u_sb[g][:, k, mm * 128 : (mm + 1) * 128],
                        et[c][:, k, :],
                        start=(k == 0),
                        stop=(k == JB - 1),
                    )
                ops_live[(c, m)] = ops

        def osb_group(c, g):
            for mm in range(4):
                m = 4 * g + mm
                osb = osb_pool.tile([128, C], F32, tag="osb", name=f"osb_{c}_{m}")
                nc.vector.tensor_mul(osb, ops_live.pop((c, m)), rden[c])
                nc.sync.dma_start(
                    out=o_dram[m * 128 : (m + 1) * 128, c * C : (c + 1) * C],
                    in_=osb,
                )

        # ---- chunk 0 ----
        exp_tr_chunk0()
        roots[0] = den_tree(0, nc.vector)
        mm_group(0, 0)
        den_mm(0)
        recip(0)
        osb_group(0, 0)
        mm_group(0, 1)
        osb_group(0, 1)
        # ---- chunk 1 prep ----
        exp_chunk(1)
        tr_chunk(1)
        roots[1] = den_tree(1, nc.gpsimd)
        mm_group(0, 2)
        osb_group(0, 2)
        mm_group(0, 3)
        osb_group(0, 3)
        den_mm(1)
        recip(1)
        # ---- chunk 1 ----
        mm_group(1, 0)
        osb_group(1, 0)
        mm_group(1, 1)
        osb_group(1, 1)
        exp_chunk(2)
        tr_chunk(2)
        roots[2] = den_tree(2, nc.gpsimd)
        mm_group(1, 2)
        osb_group(1, 2)
        mm_group(1, 3)
        osb_group(1, 3)
        den_mm(2)
        recip(2)
        # ---- chunk 2 ----
        mm_group(2, 0)
        osb_group(2, 0)
        mm_group(2, 1)
        osb_group(2, 1)
        exp_chunk(3)
        tr_chunk(3)
        roots[3] = den_tree(3, nc.gpsimd)
        mm_group(2, 2)
        osb_group(2, 2)
        mm_group(2, 3)
        osb_group(2, 3)
        den_mm(3)
        recip(3)
        # ---- chunk 3 ----
        for g in range(NG):
            mm_group(3, g)
            osb_group(3, g)

    nc.compile()
    return nc


_cached_nc = None


def _get_nc():
    global _cached_nc
    if _cached_nc is None:
        _cached_nc = _build()
    return _cached_nc


def _in_maps(u, s):
    u0 = np.asarray(u)[0].astype(np.float16)
    # [J, D] -> [g, p, b, dd]: u_r[g, p, b, dd] = u0[b*128+p, g*512+dd]
    u_r = np.ascontiguousarray(u0.reshape(JB, 128, NG, 512).transpose(2, 1, 0, 3))
    s16 = np.asarray(s).astype(np.float16)
    return [
        {
            "s_loc": np.ascontiguousarray(s16[c * TLOC : (c + 1) * TLOC]),
            "u_r": u_r,
            "ident": np.eye(128, dtype=np.float16),
            "ones_m": np.ones((128, 128), dtype=np.float16),
        }
        for c in range(NCORES)
    ]


def kernel(u, s):
    nc = _get_nc()
    in_maps = _in_maps(u, s)
    last_err = None
    for attempt in range(3):
        try:
            res = run_bass_kernel_spmd(nc, in_maps, core_ids=list(range(NCORES)))
            break
        except Exception as e:  # transient device/terminal hiccups recover on retry
            last_err = e
            time.sleep(5 * (attempt + 1))
    else:
        raise last_err
    out = np.empty((D, T), dtype=np.float32)
    for c in range(NCORES):
        out[:, c * TLOC : (c + 1) * TLOC] = res.results[c]["o_loc"]
    return out
